# revision 35
# baseline (speedup 1.0000x reference)
"""Sparse (sliding-window + sink) GQA attention block on 8 TRN2 NeuronCores.

Sharding: tensor-parallel over the 64 q-heads -> 8 q-heads (= 1 kv-head
group) per core; x replicated; wo partial outputs summed on host.

v3 dataflow (bf16 storage for DMA-heavy operands, f32 rope/scores path):
  DMA goes through one in-order ring shared by 16 engines and is
  descriptor-rate bound, so every stream uses >=2KB-per-partition-row
  chunks, issued in need-order (wq/x first, tables mid, wo late).
  x^T is DMA'd once ([128,1024] bf16 tiles, resident) and reused by both
  512-col PSUM passes of phase A.  Bias adds on ACT; RoPE on GpSimd
  (overlapped with phase A's second pass).
  Attention per head: scoresT (f32r) -> exp straight off PSUM (ACT,
  bf16) -> multiplicative 0/1 bf16 mask (GpSimd) -> sliding window
  accumulated in PSUM with two 128-col matmuls per i-block; denom row
  via ones-column of v_aug.  Epilogue: denom rows at 32*(h%4) packed in
  two [128,S] tiles; reciprocal_approx_fast (DVE); rinv broadcast by
  ones-matmul; scale into bf16 atb tiles (DVE).
  Phase D: out[i,dd] partial = atb^T wo (bf16), one [128,2880] bf16
  write per i-block; host sums partials.
"""

import numpy as np

B, S, DIM = 1, 1024, 2880
H, HKV, HD = 64, 8, 64
GROUP = H // HKV
WINDOW = 128
THETA = 150000.0
NC = 8
HL = H // NC                 # 8 local q-heads per core
EL = HL * HD                 # 512 local q-dim
DT = (DIM + 127) // 128      # 23 d-tiles (22 full + 64)
NJ = S // 128                # 8 j/i blocks

_cache = {}


def _build_module():
    import concourse.bacc as bacc
    import concourse.mybir as mybir
    import concourse.tile as tile

    f32 = mybir.dt.float32
    f32r = mybir.dt.float32r
    bf16 = mybir.dt.bfloat16
    AF = mybir.ActivationFunctionType
    OP = mybir.AluOpType

    nc = bacc.Bacc("TRN2", target_bir_lowering=False, debug=False)

    def din(name, shape, dt=f32):
        return nc.dram_tensor(name, shape, dt, kind="ExternalInput").ap()

    xT = din("xT", [DIM, S], bf16)             # x^T
    wqT = din("wqT", [128, DT * EL], bf16)     # tiled: [p, 512*t + e]
    wkvT = din("wkvT", [128, DT * 128], bf16)  # tiled: [p, 128*t + (k|v)]
    woT = din("woT", [128, 4 * DIM], bf16)     # tiled: [p, 2880*et + dd]
    qb = din("qb", [128, 4])
    kvb = din("kvb", [128, 1])
    cosq = din("cosq", [128, S])               # 0.125-scaled
    sinq = din("sinq", [128, S])               # 0.125-scaled, sign-baked
    cosk = din("cosk", [64, S])
    sinkt = din("sinkt", [64, S])
    m01 = din("m01", [128, 1024], bf16)        # 0/1 mask [std|std | std|J7]
    esink = din("esink", [128, 2])             # exp(sinks), row 32*(h%4)
    id64 = din("id64", [128, 64])              # eye(64) stacked twice
    out_d = nc.dram_tensor("out", [S, DIM], bf16, kind="ExternalOutput").ap()

    with tile.TileContext(nc) as tc:
        import contextlib
        with contextlib.ExitStack() as ctx:
            res = ctx.enter_context(tc.tile_pool(name="res", bufs=1))
            wq_sb = res.tile([128, DT * EL], bf16, tag="wq")
            wkv_sb = res.tile([128, DT * 128], bf16, tag="wkv")
            wo_sb = res.tile([128, 4 * DIM], bf16, tag="wo")
            xh_all = [res.tile([128, 1024], bf16, tag=f"xh{t}",
                               name=f"xh{t}") for t in range(DT)]
            cq_sb = res.tile([128, S], f32, tag="cq")
            sq_sb = res.tile([128, S], f32, tag="sq")
            ck_sb = res.tile([64, S], f32, tag="ck")
            sk_sb = res.tile([64, S], f32, tag="sk")
            m01_sb = res.tile([128, 1024], bf16, tag="m01")
            qb_sb = res.tile([128, 4], f32, tag="qb")
            kvb_sb = res.tile([128, 1], f32, tag="kvb")
            es_sb = res.tile([128, 2], f32, tag="es")
            id_sb = res.tile([128, 64], f32, tag="id")
            ones0 = res.tile([1, 64], f32, tag="ones0")
            onesr = res.tile([1, 64], f32, tag="onesr")
            z0 = res.tile([128, 128], f32, tag="z0")
            qT = [res.tile([128, S + 128], f32, tag=f"qT{i}", name=f"qT{i}")
                  for i in range(4)]
            kv_sb = res.tile([128, S], f32, tag="kv")
            kv2_sb = res.tile([128, S], f32, tag="kv2")
            v_sb = [res.tile([128, 65], bf16, tag=f"v{j}", name=f"v{j}")
                    for j in range(NJ)]
            at_pair = [res.tile([128, S], bf16, tag=f"at{t}", name=f"at{t}")
                       for t in range(4)]
            atb = [res.tile([128, S], bf16, tag=f"ab{t}", name=f"ab{t}")
                   for t in range(4)]
            dn_ab = [res.tile([128, S], f32, tag=f"dn{i}", name=f"dn{i}")
                     for i in range(2)]
            ri_ab = [res.tile([128, S], f32, tag=f"ri{i}", name=f"ri{i}")
                     for i in range(2)]

            nc.vector.memset(ones0[:], 1.0)
            nc.vector.tensor_copy(onesr[:].bitcast(f32r), ones0[:])
            nc.vector.memset(dn_ab[0][:], 1.0)
            nc.vector.memset(dn_ab[1][:], 1.0)
            nc.vector.memset(z0[:], 0.0)
            for et in range(4):
                nc.vector.tensor_copy(qT[et][:, S:S + 128].bitcast(f32r),
                                      z0[:])

            # ---- initial DMAs, in need-order (ring is in-order) ----
            nc.sync.dma_start(wq_sb[:, 0:EL], wqT[:, 0:EL])
            nc.sync.dma_start(wkv_sb[:, 0:128], wkvT[:, 0:128])
            nc.sync.dma_start(xh_all[0][:], xT[0:128, :])
            nc.sync.dma_start(xh_all[1][:], xT[128:256, :])
            nc.sync.dma_start(qb_sb[:], qb[:])
            nc.sync.dma_start(kvb_sb[:], kvb[:])

            # ---------------- Phase A + RoPE, per 512-col half -----------
            with tc.tile_pool(name="pqA", bufs=1, space="PSUM") as pq_pool, \
                 tc.tile_pool(name="pkvA", bufs=1, space="PSUM") as pkv_pool, \
                 tc.tile_pool(name="pvt", bufs=2, space="PSUM") as pvt_pool, \
                 tc.tile_pool(name="rope", bufs=1) as rp:
              for sc in range(2):
                pq = [pq_pool.tile([128, 512], f32, tag=f"pq{e}",
                                   name=f"pq{e}") for e in range(4)]
                pkv = pkv_pool.tile([128, 512], f32, tag="pkv")
                for t in range(DT):
                    dp = 128 if t < DT - 1 else DIM - 128 * (DT - 1)
                    if sc == 0:
                        # paced prefetch through the single DMA ring
                        if t + 2 < DT:
                            dp2 = (128 if t + 2 < DT - 1
                                   else DIM - 128 * (DT - 1))
                            nc.sync.dma_start(
                                xh_all[t + 2][:dp2, :],
                                xT[128 * (t + 2):128 * (t + 2) + dp2, :])
                        if t <= 10:
                            c0 = EL * (2 * t + 1)
                            c1 = min(EL * (2 * t + 3), DT * EL)
                            nc.sync.dma_start(wq_sb[:, c0:c1],
                                              wqT[:, c0:c1])
                        if t in (0, 4, 8):
                            k0 = 128 + 1024 * (t // 4)
                            k1 = min(k0 + 1024, DT * 128)
                            nc.sync.dma_start(wkv_sb[:, k0:k1],
                                              wkvT[:, k0:k1])
                        if t == 12:
                            nc.sync.dma_start(cq_sb[:], cosq[:])
                            nc.sync.dma_start(sq_sb[:], sinq[:])
                        if t == 14:
                            nc.sync.dma_start(ck_sb[:], cosk[:])
                            nc.sync.dma_start(sk_sb[:], sinkt[:])
                            nc.sync.dma_start(m01_sb[:], m01[:])
                            nc.sync.dma_start(es_sb[:], esink[:])
                            nc.sync.dma_start(id_sb[:], id64[:])
                    else:
                        if t in (1, 3, 5, 7, 9, 11):
                            c = t // 2
                            nc.sync.dma_start(
                                wo_sb[:, 1920 * c:1920 * (c + 1)],
                                woT[:, 1920 * c:1920 * (c + 1)])
                    rhs = xh_all[t][:dp, 512 * sc:512 * (sc + 1)]
                    st, sp = (t == 0), (t == DT - 1)
                    for et in range(4):
                        nc.tensor.matmul(
                            pq[et][:],
                            wq_sb[:dp, EL * t + 128 * et:
                                  EL * t + 128 * (et + 1)],
                            rhs, start=st, stop=sp)
                    nc.tensor.matmul(
                        pkv[:], wkv_sb[:dp, 128 * t:128 * (t + 1)],
                        rhs, start=st, stop=sp)
                hs = slice(512 * sc, 512 * (sc + 1))
                for et in range(4):
                    nc.scalar.activation(qT[et][:, hs].bitcast(f32r),
                                         pq[et][:],
                                         AF.Identity, bias=qb_sb[:, et:et + 1])
                nc.scalar.activation(kv_sb[:, hs].bitcast(f32r), pkv[:],
                                     AF.Identity, bias=kvb_sb[:, 0:1])

                # ---- RoPE for this half (GpSimd + swap DMAs), k first
                ksw = rp.tile([64, 512], f32, tag="ksw")
                nc.sync.dma_start(ksw[0:32, :], kv_sb[32:64, hs])
                nc.sync.dma_start(ksw[32:64, :], kv_sb[0:32, hs])
                ktmp = rp.tile([64, 512], f32, tag="ktmp")
                kqc = rp.tile([64, 512], f32, tag="kqc")
                nc.gpsimd.tensor_tensor(ktmp[:], ksw[:], sk_sb[:, hs],
                                        op=OP.mult)
                nc.gpsimd.tensor_tensor(kqc[:], kv_sb[0:64, hs],
                                        ck_sb[:, hs], op=OP.mult)
                nc.gpsimd.tensor_tensor(kv_sb[0:64, hs].bitcast(f32r),
                                        kqc[:], ktmp[:], op=OP.add)
                # kT copy at base 64 for odd heads
                nc.sync.dma_start(kv2_sb[64:128, hs].bitcast(f32r),
                                  kv_sb[0:64, hs].bitcast(f32r))
                # v transposes for this half (PE, tiny) -> bf16 v_sb
                for j in range(4 * sc, 4 * sc + 4):
                    pvt = pvt_pool.tile([128, 64], f32, tag="pvt")
                    nc.tensor.transpose(
                        pvt[:], kv_sb[64:128, 128 * j:128 * (j + 1)],
                        id_sb[64:128, :])
                    nc.vector.tensor_copy(v_sb[j][:, 0:64], pvt[:])
                    nc.vector.memset(v_sb[j][:, 64:65], 1.0)
                # q rope
                for et in range(4):
                    q = qT[et]
                    qsw = rp.tile([128, 512], f32, tag="qsw")
                    nc.sync.dma_start(qsw[0:32, :], q[32:64, hs])
                    nc.sync.dma_start(qsw[32:64, :], q[0:32, hs])
                    nc.sync.dma_start(qsw[64:96, :], q[96:128, hs])
                    nc.sync.dma_start(qsw[96:128, :], q[64:96, hs])
                    tmp = rp.tile([128, 512], f32, tag="tmp")
                    qc = rp.tile([128, 512], f32, tag="qc")
                    nc.gpsimd.tensor_tensor(tmp[:], qsw[:], sq_sb[:, hs],
                                            op=OP.mult)
                    nc.gpsimd.tensor_tensor(qc[:], q[:, hs], cq_sb[:, hs],
                                            op=OP.mult)
                    nc.gpsimd.tensor_tensor(q[:, hs].bitcast(f32r),
                                            qc[:], tmp[:], op=OP.add)

            # ---------------- Phase C: attention ----------------
            with tc.tile_pool(name="pbig", bufs=2, space="PSUM") as pbig_pool, \
                 tc.tile_pool(name="psc", bufs=2, space="PSUM") as ps_pool, \
                 tc.tile_pool(name="prt", bufs=2, space="PSUM") as prt_pool, \
                 tc.tile_pool(name="ee0", bufs=2) as eT0_pool, \
                 tc.tile_pool(name="eet", bufs=3) as eT_pool, \
                 tc.tile_pool(name="stg", bufs=2) as stg_pool:

                def epilogue(g):
                    # rinv for head group g (4 heads), then scale attnT
                    nc.vector.reciprocal_approx_fast(ri_ab[g][:], dn_ab[g][:])
                    for h in range(4 * g, 4 * g + 4):
                        t, r0 = h // 2, 64 * (h % 2)
                        dr = 32 * (h % 4)
                        stg = stg_pool.tile([1, S], f32, tag="stg",
                                            name=f"stg{h}")
                        nc.sync.dma_start(stg[:].bitcast(f32r),
                                          ri_ab[g][dr:dr + 1, :]
                                          .bitcast(f32r))
                        for half in range(2):
                            hs = slice(512 * half, 512 * (half + 1))
                            prt = prt_pool.tile([64, 512], f32, tag="prt")
                            nc.tensor.matmul(
                                prt[:], onesr[0:1, :].bitcast(f32r),
                                stg[0:1, hs].bitcast(f32r),
                                start=True, stop=True)
                            nc.vector.tensor_tensor(
                                atb[t][r0:r0 + 64, hs],
                                at_pair[t][r0:r0 + 64, hs],
                                prt[0:64, :], op=OP.mult)

                for h in range(HL):
                    qt = qT[h // 2]
                    r0 = 64 * (h % 2)
                    tpi = h // 2
                    kt = kv_sb if h % 2 == 0 else kv2_sb
                    pbig = pbig_pool.tile([65, S], f32, tag="pbig")
                    eS = [None] * NJ     # AP slices into pair tiles
                    for Jp in range(NJ // 2):
                        # scores for J-pair (2Jp, 2Jp+1) into one psum tile
                        ps = ps_pool.tile([128, 512], f32, tag="ps")
                        for q2 in range(2):
                            J = 2 * Jp + q2
                            nc.tensor.matmul(
                                ps[:, 256 * q2:256 * (q2 + 1)],
                                kt[r0:r0 + 64, 128 * J:128 * (J + 1)]
                                .bitcast(f32r),
                                qt[r0:r0 + 64, 128 * J:128 * J + 256]
                                .bitcast(f32r),
                                start=True, stop=True)
                        mks = (slice(0, 512) if Jp < NJ // 2 - 1
                               else slice(512, 1024))
                        eT0 = eT0_pool.tile([128, 512], bf16, tag="eT0")
                        nc.scalar.activation(eT0[:], ps[:], AF.Exp)
                        eT = eT_pool.tile([128, 512], bf16, tag="eT")
                        nc.vector.tensor_tensor(eT[:], eT0[:],
                                                m01_sb[:, mks], op=OP.mult)
                        eS[2 * Jp] = eT[:, 0:256]
                        eS[2 * Jp + 1] = eT[:, 256:512]
                        # window-accumulated attnT for i-blocks I=2Jp, 2Jp+1
                        for q2 in range(2):
                            J = 2 * Jp + q2
                            dst = pbig[:, 128 * J:128 * (J + 1)]
                            if J == 0:
                                nc.tensor.matmul(dst, v_sb[0][:, 0:65],
                                                 eS[0][:, 0:128],
                                                 start=True, stop=True)
                            else:
                                nc.tensor.matmul(dst, v_sb[J - 1][:, 0:65],
                                                 eS[J - 1][:, 128:256],
                                                 start=True, stop=False)
                                nc.tensor.matmul(dst, v_sb[J][:, 0:65],
                                                 eS[J][:, 0:128],
                                                 start=False, stop=True)
                    # drain: attnT rows -> at_pair bf16 (DVE), denom -> dn
                    nc.vector.tensor_copy(at_pair[tpi][r0:r0 + 64, :],
                                          pbig[0:64, :])
                    dr = 32 * (h % 4)
                    nc.scalar.activation(dn_ab[h // 4][dr:dr + 1, :],
                                         pbig[64:65, :], AF.Identity,
                                         bias=es_sb[dr:dr + 1,
                                                    (h // 4):(h // 4) + 1])
                    if h == 3:
                        epilogue(0)
                epilogue(1)

            # ---------------- Phase D: output projection ----------------
            NDD = 6
            DDC = DIM // NDD  # 480
            with tc.tile_pool(name="po", bufs=3, space="PSUM") as po_pool, \
                 tc.tile_pool(name="ob", bufs=2) as ob_pool:
                for it in range(NJ):
                    ob = ob_pool.tile([128, DIM], bf16, tag="ob")
                    for dd in range(NDD):
                        po = po_pool.tile([128, DDC], f32, tag="po")
                        for et in range(4):
                            nc.tensor.matmul(
                                po[:],
                                atb[et][:, 128 * it:128 * (it + 1)],
                                wo_sb[:, DIM * et + DDC * dd:
                                      DIM * et + DDC * (dd + 1)],
                                start=(et == 0), stop=(et == 3))
                        nc.scalar.activation(ob[:, DDC * dd:DDC * (dd + 1)],
                                             po[:], AF.Copy)
                        if it == NJ - 1 and dd == 2:
                            nc.sync.dma_start(
                                out_d[128 * it:128 * (it + 1), 0:3 * DDC],
                                ob[:, 0:3 * DDC])
                    if it == NJ - 1:
                        nc.sync.dma_start(
                            out_d[128 * it:128 * (it + 1), 3 * DDC:DIM],
                            ob[:, 3 * DDC:DIM])
                    else:
                        nc.sync.dma_start(out_d[128 * it:128 * (it + 1), :],
                                          ob[:])

    nc.compile()
    return nc


def _esink_layout(s8):
    out = np.zeros((128, 2), np.float32)
    for h in range(HL):
        out[32 * (h % 4), h // 4] = np.exp(np.float64(s8[h]))
    return out


def _host_prep(x, wq_w, wq_b, wk_w, wk_b, wv_w, wv_b, wo_w, wo_b, sinks):
    """Build per-core input maps (host-side sharding + layout prep)."""
    import ml_dtypes
    f = np.float32
    bf = ml_dtypes.bfloat16
    xT = np.ascontiguousarray(x.reshape(S, DIM).T).astype(bf)      # [2880,1024]

    half = HD // 2
    inv_freq = 1.0 / (THETA ** (np.arange(half, dtype=np.float64) * 2.0 / HD))
    ang = np.arange(S, dtype=np.float64)[:, None] * inv_freq       # [S, 32]
    cos_t = np.cos(ang).T.astype(f)                                # [32, S]
    sin_t = np.sin(ang).T.astype(f)
    cos64 = np.concatenate([cos_t, cos_t], 0)                      # [64, S]
    sin64 = np.concatenate([-sin_t, sin_t], 0)
    scale = np.float32(HD ** -0.5)
    cosq = np.concatenate([cos64, cos64], 0) * scale               # [128, S]
    sinq = np.concatenate([sin64, sin64], 0) * scale
    cosk = cos64
    sinkt = sin64

    jj = np.arange(128)[:, None]
    ii = np.arange(256)[None, :]
    allow_l = (jj <= ii) & (ii < 128)
    allow_r = (ii >= 128) & (jj > ii - 128)
    m_std = (allow_l | allow_r).astype(f)
    m_j7 = allow_l.astype(f)
    m01 = np.concatenate([m_std, m_std, m_std, m_j7], 1).astype(bf)

    id64 = np.tile(np.eye(64, dtype=f), (2, 1))

    def tile_T(w):  # [E, DIM] -> tiled transposed [128, DT*E]
        E = w.shape[0]
        out = np.zeros((128, DT * E), f)
        for t in range(DT):
            dp = min(128, DIM - 128 * t)
            out[:dp, E * t:E * (t + 1)] = w[:, 128 * t:128 * t + dp].T
        return out

    in_maps = []
    for c in range(NC):
        wq_c = wq_w[EL * c:EL * (c + 1)]                  # [512, 2880]
        wkv_c = np.concatenate([wk_w[HD * c:HD * (c + 1)],
                                wv_w[HD * c:HD * (c + 1)]], 0)  # [128, 2880]
        wo_c = np.ascontiguousarray(wo_w[:, EL * c:EL * (c + 1)].T)  # [512,2880]
        woT_t = np.zeros((128, 4 * DIM), f)
        for et in range(4):
            woT_t[:, DIM * et:DIM * (et + 1)] = wo_c[128 * et:128 * (et + 1)]
        in_maps.append({
            "xT": xT,
            "wqT": tile_T(wq_c).astype(bf),
            "wkvT": tile_T(wkv_c).astype(bf),
            "woT": woT_t.astype(bf),
            "qb": np.ascontiguousarray(
                wq_b[EL * c:EL * (c + 1)].reshape(4, 128).T).astype(f),
            "kvb": np.ascontiguousarray(np.concatenate(
                [wk_b[HD * c:HD * (c + 1)],
                 wv_b[HD * c:HD * (c + 1)]]).reshape(1, 128).T).astype(f),
            "cosq": cosq, "sinq": sinq, "cosk": cosk, "sinkt": sinkt,
            "m01": m01,
            "esink": _esink_layout(sinks[HL * c:HL * (c + 1)]),
            "id64": id64,
        })
    return in_maps


def run_on_hw(inputs, trace=False, **kw):
    from concourse import bass_utils
    if "nc" not in _cache:
        _cache["nc"] = _build_module()
    in_maps = _host_prep(**inputs)
    res = bass_utils.run_bass_kernel_spmd(
        _cache["nc"], in_maps, core_ids=list(range(NC)), trace=trace, **kw)
    partials = [res.results[c]["out"].astype(np.float64) for c in range(NC)]
    out = np.sum(np.stack(partials, 0), 0)
    out = (out + inputs["wo_b"].astype(np.float64)).astype(np.float32)
    return out.reshape(B, S, DIM), res


def kernel(**inputs) -> np.ndarray:
    out, _ = run_on_hw(inputs, trace=False)
    return out


# revision 41
# speedup vs baseline: 1.2350x; 1.2350x over previous
"""Sparse (sliding-window + sink) GQA attention block on 8 TRN2 NeuronCores.

Sharding: tensor-parallel over the 64 q-heads -> 8 q-heads (= 1 kv-head
group) per core; x replicated; wo partial outputs summed on host.

v3 dataflow (bf16 storage for DMA-heavy operands, f32 rope/scores path):
  DMA goes through one in-order ring shared by 16 engines and is
  descriptor-rate bound, so every stream uses >=2KB-per-partition-row
  chunks, issued in need-order (wq/x first, tables mid, wo late).
  x^T is DMA'd once ([128,1024] bf16 tiles, resident) and reused by both
  512-col PSUM passes of phase A.  Bias adds on ACT; RoPE on GpSimd
  (overlapped with phase A's second pass).
  Attention per head: scoresT (f32r) -> exp straight off PSUM (ACT,
  bf16) -> multiplicative 0/1 bf16 mask (GpSimd) -> sliding window
  accumulated in PSUM with two 128-col matmuls per i-block; denom row
  via ones-column of v_aug.  Epilogue: denom rows at 32*(h%4) packed in
  two [128,S] tiles; reciprocal_approx_fast (DVE); rinv broadcast by
  ones-matmul; scale into bf16 atb tiles (DVE).
  Phase D: out[i,dd] partial = atb^T wo (bf16), one [128,2880] bf16
  write per i-block; host sums partials.
"""

import numpy as np

B, S, DIM = 1, 1024, 2880
H, HKV, HD = 64, 8, 64
GROUP = H // HKV
WINDOW = 128
THETA = 150000.0
NC = 8
HL = H // NC                 # 8 local q-heads per core
EL = HL * HD                 # 512 local q-dim
DT = (DIM + 127) // 128      # 23 d-tiles (22 full + 64)
NJ = S // 128                # 8 j/i blocks

_cache = {}


def _build_module():
    import concourse.bacc as bacc
    import concourse.mybir as mybir
    import concourse.tile as tile

    f32 = mybir.dt.float32
    f32r = mybir.dt.float32r
    bf16 = mybir.dt.bfloat16
    AF = mybir.ActivationFunctionType
    OP = mybir.AluOpType

    nc = bacc.Bacc("TRN2", target_bir_lowering=False, debug=False)

    def din(name, shape, dt=f32):
        return nc.dram_tensor(name, shape, dt, kind="ExternalInput").ap()

    xT = din("xT", [DIM, S], bf16)             # x^T
    wqT = din("wqT", [128, DT * EL], bf16)     # tiled: [p, 512*t + e]
    wkvT = din("wkvT", [128, DT * 128], bf16)  # tiled: [p, 128*t + (k|v)]
    woT = din("woT", [128, 4 * DIM], bf16)     # tiled: [p, 2880*et + dd]
    qb = din("qb", [128, 4])
    kvb = din("kvb", [128, 1])
    cosq = din("cosq", [128, S])               # 0.125-scaled
    sinq = din("sinq", [128, S])               # 0.125-scaled, sign-baked
    cosk = din("cosk", [64, S])
    sinkt = din("sinkt", [64, S])
    m01 = din("m01", [128, 2048], bf16)        # 0/1 mask [std x4 | std x3,J7]
    esink = din("esink", [128, 2])             # exp(sinks), row 32*(h%4)
    id64 = din("id64", [128, 64])              # eye(64) stacked twice
    out_d = nc.dram_tensor("out", [S, DIM], bf16, kind="ExternalOutput").ap()

    with tile.TileContext(nc) as tc:
        import contextlib
        with contextlib.ExitStack() as ctx:
            res = ctx.enter_context(tc.tile_pool(name="res", bufs=1))
            wq_sb = res.tile([128, DT * EL], bf16, tag="wq")
            wkv_sb = res.tile([128, DT * 128], bf16, tag="wkv")
            wo_sb = res.tile([128, 4 * DIM], bf16, tag="wo")
            xh_all = [res.tile([128, 1024], bf16, tag=f"xh{t}",
                               name=f"xh{t}") for t in range(DT)]
            cq_sb = res.tile([128, S], f32, tag="cq")
            sq_sb = res.tile([128, S], f32, tag="sq")
            ck_sb = res.tile([64, S], f32, tag="ck")
            sk_sb = res.tile([64, S], f32, tag="sk")
            m01_sb = res.tile([128, 2048], bf16, tag="m01")
            qb_sb = res.tile([128, 4], f32, tag="qb")
            kvb_sb = res.tile([128, 1], f32, tag="kvb")
            es_sb = res.tile([128, 2], f32, tag="es")
            id_sb = res.tile([128, 64], f32, tag="id")
            ones0 = res.tile([1, 64], f32, tag="ones0")
            onesr = res.tile([1, 64], f32, tag="onesr")
            z0 = res.tile([128, 128], f32, tag="z0")
            qT = [res.tile([128, S + 128], f32, tag=f"qT{i}", name=f"qT{i}")
                  for i in range(4)]
            kv_sb = res.tile([128, S], f32, tag="kv")
            kv2_sb = res.tile([128, S], f32, tag="kv2")
            v_sb = [res.tile([128, 65], bf16, tag=f"v{j}", name=f"v{j}")
                    for j in range(NJ)]
            at_pair = [res.tile([128, S], bf16, tag=f"at{t}", name=f"at{t}")
                       for t in range(4)]
            atb = [res.tile([128, S], bf16, tag=f"ab{t}", name=f"ab{t}")
                   for t in range(4)]
            dn_ab = [res.tile([128, S], f32, tag=f"dn{i}", name=f"dn{i}")
                     for i in range(2)]
            ri_ab = [res.tile([128, S], f32, tag=f"ri{i}", name=f"ri{i}")
                     for i in range(2)]

            nc.vector.memset(ones0[:], 1.0)
            nc.vector.tensor_copy(onesr[:].bitcast(f32r), ones0[:])
            nc.vector.memset(dn_ab[0][:], 1.0)
            nc.vector.memset(dn_ab[1][:], 1.0)
            nc.vector.memset(z0[:], 0.0)
            for et in range(4):
                nc.vector.tensor_copy(qT[et][:, S:S + 128].bitcast(f32r),
                                      z0[:])

            # ---- initial DMAs, in need-order (ring is in-order) ----
            nc.sync.dma_start(wq_sb[:, 0:EL], wqT[:, 0:EL])
            nc.sync.dma_start(wkv_sb[:, 0:128], wkvT[:, 0:128])
            nc.sync.dma_start(xh_all[0][:], xT[0:128, :])
            nc.sync.dma_start(xh_all[1][:], xT[128:256, :])
            nc.sync.dma_start(qb_sb[:], qb[:])
            nc.sync.dma_start(kvb_sb[:], kvb[:])

            # ---------------- Phase A + RoPE, per 512-col half -----------
            with tc.tile_pool(name="pqA", bufs=1, space="PSUM") as pq_pool, \
                 tc.tile_pool(name="pkvA", bufs=1, space="PSUM") as pkv_pool, \
                 tc.tile_pool(name="pvt", bufs=2, space="PSUM") as pvt_pool, \
                 tc.tile_pool(name="rope", bufs=1) as rp:
              for sc in range(2):
                pq = [pq_pool.tile([128, 512], f32, tag=f"pq{e}",
                                   name=f"pq{e}") for e in range(4)]
                pkv = pkv_pool.tile([128, 512], f32, tag="pkv")
                for t in range(DT):
                    dp = 128 if t < DT - 1 else DIM - 128 * (DT - 1)
                    if sc == 0:
                        # paced prefetch through the single DMA ring
                        if t + 2 < DT:
                            dp2 = (128 if t + 2 < DT - 1
                                   else DIM - 128 * (DT - 1))
                            nc.sync.dma_start(
                                xh_all[t + 2][:dp2, :],
                                xT[128 * (t + 2):128 * (t + 2) + dp2, :])
                        if t <= 10:
                            c0 = EL * (2 * t + 1)
                            c1 = min(EL * (2 * t + 3), DT * EL)
                            nc.sync.dma_start(wq_sb[:, c0:c1],
                                              wqT[:, c0:c1])
                        if t in (0, 4, 8):
                            k0 = 128 + 1024 * (t // 4)
                            k1 = min(k0 + 1024, DT * 128)
                            nc.sync.dma_start(wkv_sb[:, k0:k1],
                                              wkvT[:, k0:k1])
                        if t == 19:
                            nc.sync.dma_start(cq_sb[:], cosq[:])
                            nc.sync.dma_start(sq_sb[:], sinq[:])
                        if t == 21:
                            nc.sync.dma_start(ck_sb[:], cosk[:])
                            nc.sync.dma_start(sk_sb[:], sinkt[:])
                            nc.sync.dma_start(m01_sb[:], m01[:])
                            nc.sync.dma_start(es_sb[:], esink[:])
                            nc.sync.dma_start(id_sb[:], id64[:])
                    else:
                        if t in (1, 3, 5, 7, 9, 11):
                            c = t // 2
                            nc.sync.dma_start(
                                wo_sb[:, 1920 * c:1920 * (c + 1)],
                                woT[:, 1920 * c:1920 * (c + 1)])
                    rhs = xh_all[t][:dp, 512 * sc:512 * (sc + 1)]
                    st, sp = (t == 0), (t == DT - 1)
                    for et in range(4):
                        nc.tensor.matmul(
                            pq[et][:],
                            wq_sb[:dp, EL * t + 128 * et:
                                  EL * t + 128 * (et + 1)],
                            rhs, start=st, stop=sp)
                    nc.tensor.matmul(
                        pkv[:], wkv_sb[:dp, 128 * t:128 * (t + 1)],
                        rhs, start=st, stop=sp)
                hs = slice(512 * sc, 512 * (sc + 1))
                for et in range(4):
                    nc.scalar.activation(qT[et][:, hs].bitcast(f32r),
                                         pq[et][:],
                                         AF.Identity, bias=qb_sb[:, et:et + 1])
                nc.scalar.activation(kv_sb[:, hs].bitcast(f32r), pkv[:],
                                     AF.Identity, bias=kvb_sb[:, 0:1])

                # ---- RoPE for this half (swap via DMA); k + early q tiles
                # on DVE (fast, gates attention start), late q on GpSimd
                ksw = rp.tile([64, 512], f32, tag="ksw")
                nc.sync.dma_start(ksw[0:32, :], kv_sb[32:64, hs])
                nc.sync.dma_start(ksw[32:64, :], kv_sb[0:32, hs])
                ktmp = rp.tile([64, 512], f32, tag="ktmp")
                kqc = rp.tile([64, 512], f32, tag="kqc")
                nc.vector.tensor_tensor(ktmp[:], ksw[:], sk_sb[:, hs],
                                        op=OP.mult)
                nc.vector.tensor_tensor(kqc[:], kv_sb[0:64, hs],
                                        ck_sb[:, hs], op=OP.mult)
                nc.vector.tensor_tensor(kv_sb[0:64, hs].bitcast(f32r),
                                        kqc[:], ktmp[:], op=OP.add)
                # kT copy at base 64 for odd heads
                nc.sync.dma_start(kv2_sb[64:128, hs].bitcast(f32r),
                                  kv_sb[0:64, hs].bitcast(f32r))
                # v transposes for this half (PE, tiny) -> bf16 v_sb
                for j in range(4 * sc, 4 * sc + 4):
                    pvt = pvt_pool.tile([128, 64], f32, tag="pvt")
                    nc.tensor.transpose(
                        pvt[:], kv_sb[64:128, 128 * j:128 * (j + 1)],
                        id_sb[64:128, :])
                    nc.vector.tensor_copy(v_sb[j][:, 0:64], pvt[:])
                    nc.vector.memset(v_sb[j][:, 64:65], 1.0)
                # q rope
                for et in range(4):
                    q = qT[et]
                    eng = nc.vector if et < 2 else nc.gpsimd
                    qsw = rp.tile([128, 512], f32, tag=f"qsw{et % 2}")
                    nc.sync.dma_start(qsw[0:32, :], q[32:64, hs])
                    nc.sync.dma_start(qsw[32:64, :], q[0:32, hs])
                    nc.sync.dma_start(qsw[64:96, :], q[96:128, hs])
                    nc.sync.dma_start(qsw[96:128, :], q[64:96, hs])
                    tmp = rp.tile([128, 512], f32, tag=f"tmp{et % 2}")
                    qc = rp.tile([128, 512], f32, tag=f"qc{et % 2}")
                    eng.tensor_tensor(tmp[:], qsw[:], sq_sb[:, hs],
                                      op=OP.mult)
                    eng.tensor_tensor(qc[:], q[:, hs], cq_sb[:, hs],
                                      op=OP.mult)
                    eng.tensor_tensor(q[:, hs].bitcast(f32r),
                                      qc[:], tmp[:], op=OP.add)

            # ---------------- Phase C: attention ----------------
            with tc.tile_pool(name="pbig", bufs=2, space="PSUM") as pbig_pool, \
                 tc.tile_pool(name="psc", bufs=2, space="PSUM") as ps_pool, \
                 tc.tile_pool(name="prt", bufs=2, space="PSUM") as prt_pool, \
                 tc.tile_pool(name="ee0", bufs=2) as eT0_pool, \
                 tc.tile_pool(name="eet", bufs=3) as eT_pool, \
                 tc.tile_pool(name="stg", bufs=2) as stg_pool:

                def epilogue(g):
                    # rinv for head group g (4 heads), then scale attnT
                    nc.vector.reciprocal_approx_fast(ri_ab[g][:], dn_ab[g][:])
                    for h in range(4 * g, 4 * g + 4):
                        t, r0 = h // 2, 64 * (h % 2)
                        dr = 32 * (h % 4)
                        stg = stg_pool.tile([1, S], f32, tag="stg",
                                            name=f"stg{h}")
                        nc.sync.dma_start(stg[:].bitcast(f32r),
                                          ri_ab[g][dr:dr + 1, :]
                                          .bitcast(f32r))
                        for half in range(2):
                            hs = slice(512 * half, 512 * (half + 1))
                            prt = prt_pool.tile([64, 512], f32, tag="prt")
                            nc.tensor.matmul(
                                prt[:], onesr[0:1, :].bitcast(f32r),
                                stg[0:1, hs].bitcast(f32r),
                                start=True, stop=True)
                            nc.vector.tensor_tensor(
                                atb[t][r0:r0 + 64, hs],
                                at_pair[t][r0:r0 + 64, hs],
                                prt[0:64, :], op=OP.mult)

                for h in range(HL):
                    qt = qT[h // 2]
                    r0 = 64 * (h % 2)
                    tpi = h // 2
                    kt = kv_sb if h % 2 == 0 else kv2_sb
                    pbig = pbig_pool.tile([65, S], f32, tag="pbig")
                    eS = [None] * NJ     # AP slices into quad tiles
                    eT0 = eT = None
                    for Jp in range(NJ // 2):
                        qd, qh = Jp // 2, Jp % 2   # quad index / half in quad
                        if qh == 0:
                            eT0 = eT0_pool.tile([128, 1024], bf16, tag="eT0")
                            eT = eT_pool.tile([128, 1024], bf16, tag="eT")
                        # scores for J-pair (2Jp, 2Jp+1) into one psum tile
                        ps = ps_pool.tile([128, 512], f32, tag="ps")
                        for q2 in range(2):
                            J = 2 * Jp + q2
                            nc.tensor.matmul(
                                ps[:, 256 * q2:256 * (q2 + 1)],
                                kt[r0:r0 + 64, 128 * J:128 * (J + 1)]
                                .bitcast(f32r),
                                qt[r0:r0 + 64, 128 * J:128 * J + 256]
                                .bitcast(f32r),
                                start=True, stop=True)
                        nc.scalar.activation(eT0[:, 512 * qh:512 * (qh + 1)],
                                             ps[:], AF.Exp)
                        if qh == 1:
                            # one 1024-wide 0/1 mask multiply per quad
                            nc.vector.tensor_tensor(
                                eT[:], eT0[:], m01_sb[:, 1024 * qd:
                                                      1024 * (qd + 1)],
                                op=OP.mult)
                            for J in range(4 * qd, 4 * qd + 4):
                                eS[J] = eT[:, 256 * (J % 4):
                                           256 * (J % 4) + 256]
                            # window-accumulated attnT for this quad
                            for J in range(4 * qd, 4 * qd + 4):
                                dst = pbig[:, 128 * J:128 * (J + 1)]
                                if J == 0:
                                    nc.tensor.matmul(dst, v_sb[0][:, 0:65],
                                                     eS[0][:, 0:128],
                                                     start=True, stop=True)
                                else:
                                    nc.tensor.matmul(dst,
                                                     v_sb[J - 1][:, 0:65],
                                                     eS[J - 1][:, 128:256],
                                                     start=True, stop=False)
                                    nc.tensor.matmul(dst, v_sb[J][:, 0:65],
                                                     eS[J][:, 0:128],
                                                     start=False, stop=True)
                    # drain: attnT rows -> at_pair bf16 (ACT), denom -> dn
                    nc.scalar.activation(at_pair[tpi][r0:r0 + 64, :],
                                         pbig[0:64, :], AF.Copy)
                    dr = 32 * (h % 4)
                    esap = es_sb[dr:dr + 1, (h // 4):(h // 4) + 1]
                    if h % 2 == 0:
                        nc.scalar.activation(dn_ab[h // 4][dr:dr + 1, :],
                                             pbig[64:65, :], AF.Identity,
                                             bias=esap)
                    else:
                        nc.vector.tensor_scalar_add(
                            dn_ab[h // 4][dr:dr + 1, :], pbig[64:65, :],
                            esap)
                    if h == 3:
                        epilogue(0)
                epilogue(1)

            # ---------------- Phase D: output projection ----------------
            NDD = 6
            DDC = DIM // NDD  # 480
            with tc.tile_pool(name="po", bufs=3, space="PSUM") as po_pool, \
                 tc.tile_pool(name="ob", bufs=2) as ob_pool:
                for it in range(NJ):
                    ob = ob_pool.tile([128, DIM], bf16, tag="ob")
                    for dd in range(NDD):
                        po = po_pool.tile([128, DDC], f32, tag="po")
                        for et in range(4):
                            nc.tensor.matmul(
                                po[:],
                                atb[et][:, 128 * it:128 * (it + 1)],
                                wo_sb[:, DIM * et + DDC * dd:
                                      DIM * et + DDC * (dd + 1)],
                                start=(et == 0), stop=(et == 3))
                        nc.scalar.activation(ob[:, DDC * dd:DDC * (dd + 1)],
                                             po[:], AF.Copy)
                        if it == NJ - 1 and dd == 2:
                            nc.sync.dma_start(
                                out_d[128 * it:128 * (it + 1), 0:3 * DDC],
                                ob[:, 0:3 * DDC])
                    if it == NJ - 1:
                        nc.sync.dma_start(
                            out_d[128 * it:128 * (it + 1), 3 * DDC:DIM],
                            ob[:, 3 * DDC:DIM])
                    else:
                        nc.sync.dma_start(out_d[128 * it:128 * (it + 1), :],
                                          ob[:])

    nc.compile()
    return nc


def _esink_layout(s8):
    out = np.zeros((128, 2), np.float32)
    for h in range(HL):
        out[32 * (h % 4), h // 4] = np.exp(np.float64(s8[h]))
    return out


def _host_prep(x, wq_w, wq_b, wk_w, wk_b, wv_w, wv_b, wo_w, wo_b, sinks):
    """Build per-core input maps (host-side sharding + layout prep)."""
    import ml_dtypes
    f = np.float32
    bf = ml_dtypes.bfloat16
    xT = np.ascontiguousarray(x.reshape(S, DIM).T).astype(bf)      # [2880,1024]

    half = HD // 2
    inv_freq = 1.0 / (THETA ** (np.arange(half, dtype=np.float64) * 2.0 / HD))
    ang = np.arange(S, dtype=np.float64)[:, None] * inv_freq       # [S, 32]
    cos_t = np.cos(ang).T.astype(f)                                # [32, S]
    sin_t = np.sin(ang).T.astype(f)
    cos64 = np.concatenate([cos_t, cos_t], 0)                      # [64, S]
    sin64 = np.concatenate([-sin_t, sin_t], 0)
    scale = np.float32(HD ** -0.5)
    cosq = np.concatenate([cos64, cos64], 0) * scale               # [128, S]
    sinq = np.concatenate([sin64, sin64], 0) * scale
    cosk = cos64
    sinkt = sin64

    jj = np.arange(128)[:, None]
    ii = np.arange(256)[None, :]
    allow_l = (jj <= ii) & (ii < 128)
    allow_r = (ii >= 128) & (jj > ii - 128)
    m_std = (allow_l | allow_r).astype(f)
    m_j7 = allow_l.astype(f)
    m01 = np.concatenate([m_std] * 7 + [m_j7], 1).astype(bf)

    id64 = np.tile(np.eye(64, dtype=f), (2, 1))

    def tile_T(w):  # [E, DIM] -> tiled transposed [128, DT*E]
        E = w.shape[0]
        out = np.zeros((128, DT * E), f)
        for t in range(DT):
            dp = min(128, DIM - 128 * t)
            out[:dp, E * t:E * (t + 1)] = w[:, 128 * t:128 * t + dp].T
        return out

    in_maps = []
    for c in range(NC):
        wq_c = wq_w[EL * c:EL * (c + 1)]                  # [512, 2880]
        wkv_c = np.concatenate([wk_w[HD * c:HD * (c + 1)],
                                wv_w[HD * c:HD * (c + 1)]], 0)  # [128, 2880]
        wo_c = np.ascontiguousarray(wo_w[:, EL * c:EL * (c + 1)].T)  # [512,2880]
        woT_t = np.zeros((128, 4 * DIM), f)
        for et in range(4):
            woT_t[:, DIM * et:DIM * (et + 1)] = wo_c[128 * et:128 * (et + 1)]
        in_maps.append({
            "xT": xT,
            "wqT": tile_T(wq_c).astype(bf),
            "wkvT": tile_T(wkv_c).astype(bf),
            "woT": woT_t.astype(bf),
            "qb": np.ascontiguousarray(
                wq_b[EL * c:EL * (c + 1)].reshape(4, 128).T).astype(f),
            "kvb": np.ascontiguousarray(np.concatenate(
                [wk_b[HD * c:HD * (c + 1)],
                 wv_b[HD * c:HD * (c + 1)]]).reshape(1, 128).T).astype(f),
            "cosq": cosq, "sinq": sinq, "cosk": cosk, "sinkt": sinkt,
            "m01": m01,
            "esink": _esink_layout(sinks[HL * c:HL * (c + 1)]),
            "id64": id64,
        })
    return in_maps


def run_on_hw(inputs, trace=False, **kw):
    from concourse import bass_utils
    if "nc" not in _cache:
        _cache["nc"] = _build_module()
    in_maps = _host_prep(**inputs)
    res = bass_utils.run_bass_kernel_spmd(
        _cache["nc"], in_maps, core_ids=list(range(NC)), trace=trace, **kw)
    partials = [res.results[c]["out"].astype(np.float64) for c in range(NC)]
    out = np.sum(np.stack(partials, 0), 0)
    out = (out + inputs["wo_b"].astype(np.float64)).astype(np.float32)
    return out.reshape(B, S, DIM), res


def kernel(**inputs) -> np.ndarray:
    out, _ = run_on_hw(inputs, trace=False)
    return out
